# revision 4
# baseline (speedup 1.0000x reference)
"""LossAwareMemoryBank Trainium2 kernel — v4 (fp8 DoubleRow stream, exact
rescore, single-gather endgame).

Same stream as v3 (fp8-e4m3 DoubleRow, 32 spans of 2048, ACT fp16-pack into
iota'd u32 tiles, DVE max8, top-24 candidates). Endgame restructured so each
candidate row is gathered ONCE (fp32 [raw | invnorm]):
  - per batch of 4: gather fp32 rows -> DVE STT exact rescore (accum dot) +
    ACT converts the raw rows into a resident bf16 copy Gc.
  - after sort/threshold/softmax, the weighted sum runs as 24 DVE
    scalar_tensor_tensor ops (bf16 row * w_j + f16 acc) straight from Gc —
    no ACT copies, no second gather.
  - issue order per query block: [tail stream][A-head: selection + first
    gathers][B(prev): weighted sum][A-tail: rescore/convert/softmax] so
    gathers overlap the previous block's weighted sum and Gc (single
    buffered) is reused cleanly.
"""

import os
import numpy as np
import ml_dtypes

BANK = 65536
D = 1024
B = 4096
N_CORES = 8
QPC = B // N_CORES
QB = QPC // 128
KT = D // 128
G2 = KT // 2
SPAN = 2048
NSPAN = BANK // SPAN
NTAIL = 3
NCAND = 24
NBATCH = NCAND // 4
ROWP = 1056
EPS = 1e-12
NEG = -3.0e38

LAST_RESULT = None
_CACHED = None


def _build_nc():
    import concourse.bacc as bacc
    import concourse.mybir as mybir
    import concourse.tile as tile
    import concourse.bass as bass

    f32 = mybir.dt.float32
    f16 = mybir.dt.float16
    bf16 = mybir.dt.bfloat16
    fp8 = mybir.dt.float8e4
    u32 = mybir.dt.uint32
    Alu = mybir.AluOpType
    DR = mybir.MatmulPerfMode.DoubleRow
    Act = mybir.ActivationFunctionType

    nc = bacc.Bacc("TRN2", target_bir_lowering=False, debug=False)

    qt = nc.dram_tensor("qt", [128, QB * KT * 128], fp8, kind="ExternalInput")
    mt = nc.dram_tensor("mt", [NSPAN, 128, 4 * KT * 512], fp8, kind="ExternalInput")
    qhat = nc.dram_tensor("qhat", [QPC, D], f32, kind="ExternalInput")
    maug = nc.dram_tensor("maug", [BANK, ROWP], f32, kind="ExternalInput")
    onehot = nc.dram_tensor("onehot", [QPC, NCAND], f32, kind="ExternalInput")
    out = nc.dram_tensor("out", [QPC, D], f32, kind="ExternalOutput")

    with tile.TileContext(nc) as tc:
        with (
            tc.tile_pool(name="constp", bufs=1) as constp,
            tc.tile_pool(name="streamp", bufs=4) as streamp,
            tc.tile_pool(name="packp", bufs=1) as packp,
            tc.tile_pool(name="psump", bufs=2, space="PSUM") as psump,
            tc.tile_pool(name="endp", bufs=2) as endp,
        ):
            qt_sb = constp.tile([128, QB, KT, 128], fp8)
            nc.sync.dma_start(
                qt_sb[:], qt[:].rearrange("p (q k c) -> p q k c", q=QB, k=KT)
            )
            addend = constp.tile([128, NSPAN * 8], u32)
            nc.gpsimd.iota(addend[:], [[SPAN, NSPAN], [0, 8]], channel_multiplier=0)
            mask_lo = constp.tile([128, 1], u32)
            nc.vector.memset(mask_lo[:], 0x0000FFFF)

            NPK = 2
            pks = []
            for i in range(NPK):
                pk = packp.tile([128, SPAN], u32, name=f"pk{i}", tag=f"pk{i}")
                nc.gpsimd.iota(pk[:], [[1, SPAN]], channel_multiplier=0)
                pks.append(pk)

            def pk_hi(pk):
                return pk.bitcast(f16).rearrange("p (w two) -> p w two", two=2)[:, :, 1]

            cands = [
                packp.tile([128, NSPAN * 8], f32, name=f"cand{qb}", tag=f"cand{qb}")
                for qb in range(QB)
            ]

            def stream_block(s, qb, mt_sb, pk_i):
                ps = psump.tile([128, 4, 512], f32, tag="ps", name="ps")
                for g in range(G2):
                    for cc in range(4):
                        nc.tensor.matmul(
                            out=ps[:, cc, :],
                            lhsT=qt_sb[:, qb, 2 * g : 2 * g + 2, :],
                            rhs=mt_sb[:, cc, 2 * g : 2 * g + 2, :],
                            start=(g == 0),
                            stop=(g == G2 - 1),
                            perf_mode=DR,
                        )
                pk = pks[pk_i % NPK]
                nc.scalar.activation(
                    out=pk_hi(pk),
                    in_=ps.rearrange("p c n -> p (c n)"),
                    func=Act.Copy,
                )
                nc.vector.max(
                    out=cands[qb][:, s * 8 : (s + 1) * 8],
                    in_=pk.bitcast(f32),
                )

            def top_rounds(dst, src_vals, scratch, nc_cand):
                nc.vector.max(out=dst[:, 0:8], in_=src_vals[:])
                cur = src_vals
                for r in range(1, nc_cand // 8):
                    nxt = scratch[r - 1]
                    nc.vector.match_replace(
                        out=nxt[:],
                        in_to_replace=dst[:, 8 * r - 8 : 8 * r],
                        in_values=cur[:],
                        imm_value=NEG,
                    )
                    nc.vector.max(out=dst[:, 8 * r : 8 * r + 8], in_=nxt[:])
                    cur = nxt

            def gather_batch(idx24, b):
                gbuf = endp.tile([128, 4, ROWP], f32, tag="gbuf", bufs=2, name="gbuf")
                for jj in range(4):
                    nc.gpsimd.indirect_dma_start(
                        out=gbuf[:, jj, :],
                        out_offset=None,
                        in_=maug[:, :],
                        in_offset=bass.IndirectOffsetOnAxis(
                            ap=idx24[:, 4 * b + jj : 4 * b + jj + 1], axis=0
                        ),
                    )
                return gbuf

            def endgame_a_head(qb):
                """Selection + first gathers."""
                cand = cands[qb]
                cu = cand.bitcast(u32)
                nc.vector.tensor_tensor(out=cu, in0=cu, in1=addend[:], op=Alu.bitwise_or)

                cand24 = endp.tile([128, NCAND], f32, tag="cand24", name="cand24")
                poisA = endp.tile([128, NSPAN * 8], f32, tag="poisA", bufs=1, name="poisA")
                poisB = endp.tile([128, NSPAN * 8], f32, tag="poisB", bufs=1, name="poisB")
                top_rounds(cand24, cand, [poisA, poisB], NCAND)

                idx24 = endp.tile([128, NCAND], u32, tag="idx24", name="idx24")
                nc.vector.tensor_scalar(
                    idx24[:], cand24.bitcast(u32), mask_lo[:, 0:1], None,
                    Alu.bitwise_and,
                )

                qh = endp.tile([128, D], f32, tag="qh", bufs=1, name="qh")
                nc.sync.dma_start(qh[:], qhat[qb * 128 : (qb + 1) * 128, :])
                oh = endp.tile([128, NCAND], f32, tag="oh", name="oh")
                nc.sync.dma_start(oh[:], onehot[qb * 128 : (qb + 1) * 128, :])

                gbufs = [gather_batch(idx24, 0), gather_batch(idx24, 1)]
                return idx24, qh, oh, gbufs

            def endgame_a_tail(qb, idx24, qh, oh, gbufs, Gc):
                """Rescore (exact fp32) + convert rows to bf16 Gc + weights."""
                s_cos = endp.tile([128, NCAND], f32, tag="s_cos", name="s_cos")
                svec = endp.tile([128, NCAND], f32, tag="svec", name="svec")
                prod = endp.tile([128, D], f32, tag="prod", bufs=1, name="prod")
                for b in range(NBATCH):
                    gbuf = gbufs[b] if b < 2 else gather_batch(idx24, b)
                    for jj in range(4):
                        nc.vector.scalar_tensor_tensor(
                            out=prod[:],
                            in0=qh[:],
                            scalar=1.0,
                            in1=gbuf[:, jj, 0:D],
                            op0=Alu.mult,
                            op1=Alu.mult,
                            accum_out=svec[:, 4 * b + jj : 4 * b + jj + 1],
                        )
                    for jj in range(4):
                        nc.scalar.activation(
                            out=Gc[:, 4 * b + jj, :],
                            in_=gbuf[:, jj, 0:D],
                            func=Act.Copy,
                        )
                    # (Gc holds f16 raw rows; |memory| values fit f16 range)
                    nc.vector.tensor_tensor(
                        out=s_cos[:, 4 * b : 4 * b + 4],
                        in0=svec[:, 4 * b : 4 * b + 4],
                        in1=gbuf[:, :, D : D + 1].opt(),
                        op=Alu.mult,
                    )

                sort24 = endp.tile([128, NCAND], f32, tag="sort24", name="sort24")
                sc1 = endp.tile([128, NCAND], f32, tag="sc1", name="sc1")
                sc2 = endp.tile([128, NCAND], f32, tag="sc2", name="sc2")
                top_rounds(sort24, s_cos, [sc1, sc2], NCAND)

                thr = endp.tile([128, 1], f32, tag="thr", name="thr")
                scr = endp.tile([128, NCAND], f32, tag="scr", name="scr")
                nc.vector.scalar_tensor_tensor(
                    out=scr[:], in0=sort24[:], scalar=1.0, in1=oh[:],
                    op0=Alu.mult, op1=Alu.mult, accum_out=thr[:, 0:1],
                )
                maxneg = endp.tile([128, 1], f32, tag="maxneg", name="maxneg")
                nc.vector.tensor_scalar_mul(maxneg[:], sort24[:, 0:1], -1.0)

                e = endp.tile([128, NCAND], f32, tag="e", name="e")
                nc.scalar.activation(
                    out=e[:], in_=s_cos[:], func=Act.Exp,
                    bias=maxneg[:, 0:1], scale=1.0,
                )
                ge = endp.tile([128, NCAND], f32, tag="ge", name="ge")
                nc.vector.tensor_scalar(ge[:], s_cos[:], thr[:, 0:1], None, Alu.is_ge)
                w = endp.tile([128, NCAND], f32, tag="w", name="w")
                denom = endp.tile([128, 1], f32, tag="denom", name="denom")
                nc.vector.scalar_tensor_tensor(
                    out=w[:], in0=e[:], scalar=1.0, in1=ge[:],
                    op0=Alu.mult, op1=Alu.mult, accum_out=denom[:, 0:1],
                )
                winv = endp.tile([128, 1], f32, tag="winv", name="winv")
                nc.vector.reciprocal(winv[:], denom[:])
                # f16 copy of the weights so the wsum STT links are all-16-bit
                wf = endp.tile([128, NCAND], f16, tag="wf", name="wf")
                nc.vector.tensor_scalar_mul(wf[:], w[:], 1.0)
                return w, wf, winv

            NSTT = 12  # candidates accumulated via DVE STT; rest via ACT+TT

            def endgame_b(qb, Gc, w, wf, winv):
                """Weighted sum split between DVE STT links (all-f16) and
                ACT scale-copy + DVE 2x TT adds."""
                acc0 = endp.tile([128, D], f16, tag="acc0", bufs=1, name="acc0")
                acc1 = endp.tile([128, D], f16, tag="acc1", bufs=1, name="acc1")
                accs = [acc0, acc1]
                nc.scalar.activation(
                    out=acc0[:], in_=Gc[:, 0, :], func=Act.Copy, scale=w[:, 0:1]
                )
                for j in range(1, NCAND):
                    if j <= NSTT:
                        nc.vector.scalar_tensor_tensor(
                            out=accs[j % 2][:],
                            in0=Gc[:, j, :],
                            scalar=wf[:, j : j + 1],
                            in1=accs[(j - 1) % 2][:],
                            op0=Alu.mult,
                            op1=Alu.add,
                        )
                    else:
                        tmp = endp.tile([128, D], f16, tag="atmp", bufs=3, name="tmp")
                        nc.scalar.activation(
                            out=tmp[:], in_=Gc[:, j, :], func=Act.Copy,
                            scale=w[:, j : j + 1],
                        )
                        nc.vector.tensor_tensor(
                            out=accs[j % 2][:], in0=tmp[:],
                            in1=accs[(j - 1) % 2][:], op=Alu.add,
                        )
                final = endp.tile([128, D], f32, tag="final", bufs=1, name="final")
                nc.scalar.activation(
                    out=final[:], in_=accs[(NCAND - 1) % 2][:], func=Act.Copy,
                    scale=winv[:, 0:1],
                )
                nc.sync.dma_start(out[qb * 128 : (qb + 1) * 128, :], final[:])

            # ---- main stream ----
            pk_i = 0
            for s in range(NSPAN - NTAIL):
                mt_sb = streamp.tile([128, 4, KT, 512], fp8, tag="mt_sb", name="mt_sb")
                nc.sync.dma_start(
                    mt_sb[:], mt[s].rearrange("p (c k n) -> p c k n", c=4, k=KT)
                )
                for qb in range(QB):
                    stream_block(s, qb, mt_sb, pk_i)
                    pk_i += 1

            # ---- tail ----
            tails = []
            for t in range(NTAIL):
                s = NSPAN - NTAIL + t
                mt_sb = streamp.tile([128, 4, KT, 512], fp8, tag="mt_sb", name="mt_sb")
                nc.sync.dma_start(
                    mt_sb[:], mt[s].rearrange("p (c k n) -> p c k n", c=4, k=KT)
                )
                tails.append((s, mt_sb))

            Gc = endp.tile([128, NCAND, D], f16, tag="Gc", bufs=1, name="Gc")
            pendingB = None
            for qb in range(QB):
                for s, mt_sb in tails:
                    stream_block(s, qb, mt_sb, pk_i)
                    pk_i += 1
                idx24, qh, oh, gbufs = endgame_a_head(qb)
                if pendingB is not None:
                    endgame_b(*pendingB)
                w, wf, winv = endgame_a_tail(qb, idx24, qh, oh, gbufs, Gc)
                pendingB = (qb, Gc, w, wf, winv)
            endgame_b(*pendingB)

    nc.compile()
    return nc


def _host_prep(query, predictions, memory):
    q = np.asarray(query, dtype=np.float32)
    p = np.asarray(predictions, dtype=np.float32)
    m = np.asarray(memory, dtype=np.float32)

    qn = np.sqrt(np.sum(q ** 2, axis=1, dtype=np.float32))
    qhat = q / np.maximum(qn, np.float32(EPS))[:, None]
    mn = np.sqrt(np.sum(m ** 2, axis=1, dtype=np.float32))
    minv = (np.float32(1.0) / np.maximum(mn, np.float32(EPS))).astype(np.float32)
    mhat = m * minv[:, None]

    probs = np.float32(1.0) / (np.float32(1.0) + np.exp(-p, dtype=np.float32))
    conf = np.mean(np.abs(probs - np.float32(0.5)), axis=1, dtype=np.float32)
    k_f = np.float32(1.0) + np.float32(9.0) * (np.float32(1.0) - conf)
    k_i = np.minimum(np.floor(k_f).astype(np.int32), BANK)
    onehot = np.zeros((B, NCAND), dtype=np.float32)
    onehot[np.arange(B), np.clip(k_i - 1, 0, NCAND - 1)] = 1.0

    m8 = mhat.astype(ml_dtypes.float8_e4m3)
    mt = (
        m8.reshape(NSPAN, 4, 512, KT, 128)
        .transpose(0, 4, 1, 3, 2)
        .reshape(NSPAN, 128, 4 * KT * 512)
        .copy()
    )
    maug = np.zeros((BANK, ROWP), dtype=np.float32)
    maug[:, :D] = m
    maug[:, D] = minv

    q8 = qhat.astype(ml_dtypes.float8_e4m3)

    per_core = []
    for core in range(N_CORES):
        qs = slice(core * QPC, (core + 1) * QPC)
        qt_c = (
            q8[qs]
            .reshape(QB, 128, KT, 128)
            .transpose(3, 0, 2, 1)
            .reshape(128, QB * KT * 128)
            .copy()
        )
        per_core.append(
            {
                "qt": qt_c,
                "mt": mt,
                "qhat": np.ascontiguousarray(qhat[qs]),
                "maug": maug,
                "onehot": np.ascontiguousarray(onehot[qs]),
            }
        )
    return per_core


def kernel(query, predictions, memory):
    global _CACHED, LAST_RESULT
    from concourse.bass_utils import run_bass_kernel_spmd

    if _CACHED is None:
        _CACHED = _build_nc()
    nc = _CACHED

    in_maps = _host_prep(query, predictions, memory)
    trace = os.environ.get("CC_KERNEL_TRACE", "0") == "1"
    res = run_bass_kernel_spmd(
        nc,
        in_maps,
        core_ids=list(range(N_CORES)),
        trace=trace,
    )
    LAST_RESULT = res
    return np.concatenate([r["out"] for r in res.results], axis=0)


# revision 5
# speedup vs baseline: 1.0083x; 1.0083x over previous
"""LossAwareMemoryBank Trainium2 kernel — v5 (fp8 DoubleRow stream, exact
rescore, single-gather endgame, split f16 weighted sum).

Same stream as v3 (fp8-e4m3 DoubleRow, 32 spans of 2048, ACT fp16-pack into
iota'd u32 tiles, DVE max8, top-24 candidates). Endgame restructured so each
candidate row is gathered ONCE (fp32 [raw | invnorm]):
  - per batch of 4: gather fp32 rows -> DVE STT exact rescore (accum dot) +
    ACT converts the raw rows into a resident f16 copy Gc.
  - after sort/threshold/softmax, the weighted sum is split: 12 DVE
    scalar_tensor_tensor links (all-f16) plus 11 ACT scale-copies with DVE
    2x f16 tensor_tensor adds — no second gather.
  - issue order per query block: [tail stream][A-head: selection + first
    gathers][B(prev): weighted sum][A-tail: rescore/convert/softmax] so
    gathers overlap the previous block's weighted sum and Gc (single
    buffered) is reused cleanly.
"""

import os
import numpy as np
import ml_dtypes

BANK = 65536
D = 1024
B = 4096
N_CORES = 8
QPC = B // N_CORES
QB = QPC // 128
KT = D // 128
G2 = KT // 2
SPAN = 2048
NSPAN = BANK // SPAN
NTAIL = 3
NCAND = 24
NBATCH = NCAND // 4
ROWP = 1056
EPS = 1e-12
NEG = -3.0e38

LAST_RESULT = None
_CACHED = None


def _build_nc():
    import concourse.bacc as bacc
    import concourse.mybir as mybir
    import concourse.tile as tile
    import concourse.bass as bass

    f32 = mybir.dt.float32
    f16 = mybir.dt.float16
    bf16 = mybir.dt.bfloat16
    fp8 = mybir.dt.float8e4
    u32 = mybir.dt.uint32
    Alu = mybir.AluOpType
    DR = mybir.MatmulPerfMode.DoubleRow
    Act = mybir.ActivationFunctionType

    nc = bacc.Bacc("TRN2", target_bir_lowering=False, debug=False)

    qt = nc.dram_tensor("qt", [128, QB * KT * 128], fp8, kind="ExternalInput")
    mt = nc.dram_tensor("mt", [NSPAN, 128, 4 * KT * 512], fp8, kind="ExternalInput")
    qhat = nc.dram_tensor("qhat", [QPC, D], f32, kind="ExternalInput")
    maug = nc.dram_tensor("maug", [BANK, ROWP], f32, kind="ExternalInput")
    onehot = nc.dram_tensor("onehot", [QPC, NCAND], f32, kind="ExternalInput")
    out = nc.dram_tensor("out", [QPC, D], f32, kind="ExternalOutput")

    with tile.TileContext(nc) as tc:
        with (
            tc.tile_pool(name="constp", bufs=1) as constp,
            tc.tile_pool(name="streamp", bufs=4) as streamp,
            tc.tile_pool(name="packp", bufs=1) as packp,
            tc.tile_pool(name="psump", bufs=2, space="PSUM") as psump,
            tc.tile_pool(name="endp", bufs=2) as endp,
        ):
            qt_sb = constp.tile([128, QB, KT, 128], fp8)
            nc.sync.dma_start(
                qt_sb[:], qt[:].rearrange("p (q k c) -> p q k c", q=QB, k=KT)
            )
            addend = constp.tile([128, NSPAN * 8], u32)
            nc.gpsimd.iota(addend[:], [[SPAN, NSPAN], [0, 8]], channel_multiplier=0)
            mask_lo = constp.tile([128, 1], u32)
            nc.vector.memset(mask_lo[:], 0x0000FFFF)

            NPK = 2
            pks = []
            for i in range(NPK):
                pk = packp.tile([128, SPAN], u32, name=f"pk{i}", tag=f"pk{i}")
                nc.gpsimd.iota(pk[:], [[1, SPAN]], channel_multiplier=0)
                pks.append(pk)

            def pk_hi(pk):
                return pk.bitcast(f16).rearrange("p (w two) -> p w two", two=2)[:, :, 1]

            cands = [
                packp.tile([128, NSPAN * 8], f32, name=f"cand{qb}", tag=f"cand{qb}")
                for qb in range(QB)
            ]

            def stream_block(s, qb, mt_sb, pk_i):
                ps = psump.tile([128, 4, 512], f32, tag="ps", name="ps")
                for g in range(G2):
                    for cc in range(4):
                        nc.tensor.matmul(
                            out=ps[:, cc, :],
                            lhsT=qt_sb[:, qb, 2 * g : 2 * g + 2, :],
                            rhs=mt_sb[:, cc, 2 * g : 2 * g + 2, :],
                            start=(g == 0),
                            stop=(g == G2 - 1),
                            perf_mode=DR,
                        )
                pk = pks[pk_i % NPK]
                nc.scalar.activation(
                    out=pk_hi(pk),
                    in_=ps.rearrange("p c n -> p (c n)"),
                    func=Act.Copy,
                )
                nc.vector.max(
                    out=cands[qb][:, s * 8 : (s + 1) * 8],
                    in_=pk.bitcast(f32),
                )

            def top_rounds(dst, src_vals, scratch, nc_cand):
                nc.vector.max(out=dst[:, 0:8], in_=src_vals[:])
                cur = src_vals
                for r in range(1, nc_cand // 8):
                    nxt = scratch[r - 1]
                    nc.vector.match_replace(
                        out=nxt[:],
                        in_to_replace=dst[:, 8 * r - 8 : 8 * r],
                        in_values=cur[:],
                        imm_value=NEG,
                    )
                    nc.vector.max(out=dst[:, 8 * r : 8 * r + 8], in_=nxt[:])
                    cur = nxt

            def gather_batch(idx24, b):
                gbuf = endp.tile([128, 4, ROWP], f32, tag="gbuf", bufs=2, name="gbuf")
                for jj in range(4):
                    nc.gpsimd.indirect_dma_start(
                        out=gbuf[:, jj, :],
                        out_offset=None,
                        in_=maug[:, :],
                        in_offset=bass.IndirectOffsetOnAxis(
                            ap=idx24[:, 4 * b + jj : 4 * b + jj + 1], axis=0
                        ),
                    )
                return gbuf

            def endgame_a_head(qb):
                """Selection + first gathers."""
                cand = cands[qb]
                cu = cand.bitcast(u32)
                nc.vector.tensor_tensor(out=cu, in0=cu, in1=addend[:], op=Alu.bitwise_or)

                cand24 = endp.tile([128, NCAND], f32, tag="cand24", name="cand24")
                poisA = endp.tile([128, NSPAN * 8], f32, tag="poisA", bufs=1, name="poisA")
                poisB = endp.tile([128, NSPAN * 8], f32, tag="poisB", bufs=1, name="poisB")
                top_rounds(cand24, cand, [poisA, poisB], NCAND)

                idx24 = endp.tile([128, NCAND], u32, tag="idx24", name="idx24")
                nc.vector.tensor_scalar(
                    idx24[:], cand24.bitcast(u32), mask_lo[:, 0:1], None,
                    Alu.bitwise_and,
                )

                qh = endp.tile([128, D], f32, tag="qh", bufs=1, name="qh")
                nc.sync.dma_start(qh[:], qhat[qb * 128 : (qb + 1) * 128, :])
                oh = endp.tile([128, NCAND], f32, tag="oh", name="oh")
                nc.sync.dma_start(oh[:], onehot[qb * 128 : (qb + 1) * 128, :])

                gbufs = [gather_batch(idx24, 0), gather_batch(idx24, 1)]
                return idx24, qh, oh, gbufs

            def endgame_a_tail(qb, idx24, qh, oh, gbufs, Gc):
                """Rescore (exact fp32) + convert rows to bf16 Gc + weights."""
                s_cos = endp.tile([128, NCAND], f32, tag="s_cos", name="s_cos")
                svec = endp.tile([128, NCAND], f32, tag="svec", name="svec")
                prod = endp.tile([128, D], f32, tag="prod", bufs=1, name="prod")
                for b in range(NBATCH):
                    gbuf = gbufs[b] if b < 2 else gather_batch(idx24, b)
                    for jj in range(4):
                        nc.vector.scalar_tensor_tensor(
                            out=prod[:],
                            in0=qh[:],
                            scalar=1.0,
                            in1=gbuf[:, jj, 0:D],
                            op0=Alu.mult,
                            op1=Alu.mult,
                            accum_out=svec[:, 4 * b + jj : 4 * b + jj + 1],
                        )
                    for jj in range(4):
                        nc.scalar.activation(
                            out=Gc[:, 4 * b + jj, :],
                            in_=gbuf[:, jj, 0:D],
                            func=Act.Copy,
                        )
                    # (Gc holds f16 raw rows; |memory| values fit f16 range)
                    nc.vector.tensor_tensor(
                        out=s_cos[:, 4 * b : 4 * b + 4],
                        in0=svec[:, 4 * b : 4 * b + 4],
                        in1=gbuf[:, :, D : D + 1].opt(),
                        op=Alu.mult,
                    )

                sort24 = endp.tile([128, NCAND], f32, tag="sort24", name="sort24")
                sc1 = endp.tile([128, NCAND], f32, tag="sc1", name="sc1")
                sc2 = endp.tile([128, NCAND], f32, tag="sc2", name="sc2")
                top_rounds(sort24, s_cos, [sc1, sc2], NCAND)

                thr = endp.tile([128, 1], f32, tag="thr", name="thr")
                scr = endp.tile([128, NCAND], f32, tag="scr", name="scr")
                nc.vector.scalar_tensor_tensor(
                    out=scr[:], in0=sort24[:], scalar=1.0, in1=oh[:],
                    op0=Alu.mult, op1=Alu.mult, accum_out=thr[:, 0:1],
                )
                maxneg = endp.tile([128, 1], f32, tag="maxneg", name="maxneg")
                nc.vector.tensor_scalar_mul(maxneg[:], sort24[:, 0:1], -1.0)

                e = endp.tile([128, NCAND], f32, tag="e", name="e")
                nc.scalar.activation(
                    out=e[:], in_=s_cos[:], func=Act.Exp,
                    bias=maxneg[:, 0:1], scale=1.0,
                )
                ge = endp.tile([128, NCAND], f32, tag="ge", name="ge")
                nc.vector.tensor_scalar(ge[:], s_cos[:], thr[:, 0:1], None, Alu.is_ge)
                w = endp.tile([128, NCAND], f32, tag="w", name="w")
                denom = endp.tile([128, 1], f32, tag="denom", name="denom")
                nc.vector.scalar_tensor_tensor(
                    out=w[:], in0=e[:], scalar=1.0, in1=ge[:],
                    op0=Alu.mult, op1=Alu.mult, accum_out=denom[:, 0:1],
                )
                winv = endp.tile([128, 1], f32, tag="winv", name="winv")
                nc.vector.reciprocal(winv[:], denom[:])
                # f16 copy of the weights so the wsum STT links are all-16-bit
                wf = endp.tile([128, NCAND], f16, tag="wf", name="wf")
                nc.vector.tensor_scalar_mul(wf[:], w[:], 1.0)
                return w, wf, winv

            NSTT = 12  # candidates accumulated via DVE STT; rest via ACT+TT

            def endgame_b(qb, Gc, w, wf, winv):
                """Weighted sum split between DVE STT links (all-f16) and
                ACT scale-copy + DVE 2x TT adds."""
                acc0 = endp.tile([128, D], f16, tag="acc0", bufs=1, name="acc0")
                acc1 = endp.tile([128, D], f16, tag="acc1", bufs=1, name="acc1")
                accs = [acc0, acc1]
                nc.scalar.activation(
                    out=acc0[:], in_=Gc[:, 0, :], func=Act.Copy, scale=w[:, 0:1]
                )
                for j in range(1, NCAND):
                    if j <= NSTT:
                        nc.vector.scalar_tensor_tensor(
                            out=accs[j % 2][:],
                            in0=Gc[:, j, :],
                            scalar=wf[:, j : j + 1],
                            in1=accs[(j - 1) % 2][:],
                            op0=Alu.mult,
                            op1=Alu.add,
                        )
                    else:
                        tmp = endp.tile([128, D], f16, tag="atmp", bufs=3, name="tmp")
                        nc.scalar.activation(
                            out=tmp[:], in_=Gc[:, j, :], func=Act.Copy,
                            scale=w[:, j : j + 1],
                        )
                        nc.vector.tensor_tensor(
                            out=accs[j % 2][:], in0=tmp[:],
                            in1=accs[(j - 1) % 2][:], op=Alu.add,
                        )
                final = endp.tile([128, D], f32, tag="final", bufs=1, name="final")
                nc.scalar.activation(
                    out=final[:], in_=accs[(NCAND - 1) % 2][:], func=Act.Copy,
                    scale=winv[:, 0:1],
                )
                nc.sync.dma_start(out[qb * 128 : (qb + 1) * 128, :], final[:])

            # ---- main stream ----
            pk_i = 0
            for s in range(NSPAN - NTAIL):
                mt_sb = streamp.tile([128, 4, KT, 512], fp8, tag="mt_sb", name="mt_sb")
                nc.sync.dma_start(
                    mt_sb[:], mt[s].rearrange("p (c k n) -> p c k n", c=4, k=KT)
                )
                for qb in range(QB):
                    stream_block(s, qb, mt_sb, pk_i)
                    pk_i += 1

            # ---- tail ----
            tails = []
            for t in range(NTAIL):
                s = NSPAN - NTAIL + t
                mt_sb = streamp.tile([128, 4, KT, 512], fp8, tag="mt_sb", name="mt_sb")
                nc.sync.dma_start(
                    mt_sb[:], mt[s].rearrange("p (c k n) -> p c k n", c=4, k=KT)
                )
                tails.append((s, mt_sb))

            Gc = endp.tile([128, NCAND, D], f16, tag="Gc", bufs=1, name="Gc")
            pendingB = None
            for qb in range(QB):
                for s, mt_sb in tails:
                    stream_block(s, qb, mt_sb, pk_i)
                    pk_i += 1
                idx24, qh, oh, gbufs = endgame_a_head(qb)
                if pendingB is not None:
                    endgame_b(*pendingB)
                w, wf, winv = endgame_a_tail(qb, idx24, qh, oh, gbufs, Gc)
                pendingB = (qb, Gc, w, wf, winv)
            endgame_b(*pendingB)

    nc.compile()
    return nc


def _host_prep(query, predictions, memory):
    q = np.asarray(query, dtype=np.float32)
    p = np.asarray(predictions, dtype=np.float32)
    m = np.asarray(memory, dtype=np.float32)

    qn = np.sqrt(np.sum(q ** 2, axis=1, dtype=np.float32))
    qhat = q / np.maximum(qn, np.float32(EPS))[:, None]
    mn = np.sqrt(np.sum(m ** 2, axis=1, dtype=np.float32))
    minv = (np.float32(1.0) / np.maximum(mn, np.float32(EPS))).astype(np.float32)
    mhat = m * minv[:, None]

    probs = np.float32(1.0) / (np.float32(1.0) + np.exp(-p, dtype=np.float32))
    conf = np.mean(np.abs(probs - np.float32(0.5)), axis=1, dtype=np.float32)
    k_f = np.float32(1.0) + np.float32(9.0) * (np.float32(1.0) - conf)
    k_i = np.minimum(np.floor(k_f).astype(np.int32), BANK)
    onehot = np.zeros((B, NCAND), dtype=np.float32)
    onehot[np.arange(B), np.clip(k_i - 1, 0, NCAND - 1)] = 1.0

    m8 = mhat.astype(ml_dtypes.float8_e4m3)
    mt = (
        m8.reshape(NSPAN, 4, 512, KT, 128)
        .transpose(0, 4, 1, 3, 2)
        .reshape(NSPAN, 128, 4 * KT * 512)
        .copy()
    )
    maug = np.zeros((BANK, ROWP), dtype=np.float32)
    maug[:, :D] = m
    maug[:, D] = minv

    q8 = qhat.astype(ml_dtypes.float8_e4m3)

    per_core = []
    for core in range(N_CORES):
        qs = slice(core * QPC, (core + 1) * QPC)
        qt_c = (
            q8[qs]
            .reshape(QB, 128, KT, 128)
            .transpose(3, 0, 2, 1)
            .reshape(128, QB * KT * 128)
            .copy()
        )
        per_core.append(
            {
                "qt": qt_c,
                "mt": mt,
                "qhat": np.ascontiguousarray(qhat[qs]),
                "maug": maug,
                "onehot": np.ascontiguousarray(onehot[qs]),
            }
        )
    return per_core


def kernel(query, predictions, memory):
    global _CACHED, LAST_RESULT
    from concourse.bass_utils import run_bass_kernel_spmd

    if _CACHED is None:
        _CACHED = _build_nc()
    nc = _CACHED

    in_maps = _host_prep(query, predictions, memory)
    trace = os.environ.get("CC_KERNEL_TRACE", "0") == "1"
    res = run_bass_kernel_spmd(
        nc,
        in_maps,
        core_ids=list(range(N_CORES)),
        trace=trace,
    )
    LAST_RESULT = res
    return np.concatenate([r["out"] for r in res.results], axis=0)
